# revision 15
# baseline (speedup 1.0000x reference)
"""Trainium2 Bass kernel for nn_AutoEncIndex_33887291965861 (topk_masking).

Reference computation:
    soft  = softmax((mat + noise) / temperature)            [training w/ gumbel]
    index = top_k(soft, J).indices                          (full descending sort)
    sel   = greedy row-by-row assignment (first J rows pick the best
            still-unused joint; later rows pick their argmax)
    out   = stop_grad(one_hot(sel)) - stop_grad(mat) + mat

Key facts used here:
  * (0 - m) + m == +0.0 exactly in IEEE fp32, so the output is an exact
    one-hot matrix except the selected entry is (1 - m) + m, which rounds
    back to exactly 1.0 for these inputs (measured rel err 0.0).
  * softmax and /temperature are strictly monotone per row, so the selection
    order is the order of w = mat + noise (fp32), with lowest-index
    tie-breaking (lax.top_k semantics == vector-engine max_index semantics).
  * The greedy pass over the first J rows selects, for row r, the
    still-available joint with the highest w[r] value (proof: the first
    available joint in row r's descending order always sits within the
    first r+1 positions by pigeonhole, which is exactly the cols<=r window
    the reference uses).  Rows >= J just take their argmax.

Device kernel (SPMD over 8 cores, row-sharded, 4096 rows/core):
  stream 2 MB chunks (256 rows of [mat;noise] as 128 partitions x 2 row
  segments, 6-deep double buffering) on the SP HWDGE ring — a single ring
  saturates the per-core DMA bus (~360 GB/s).  w = mat + noise on the
  gpsimd (Pool) engine; the vector engine does the per-segment max (one
  segmented tensor_reduce per chunk) and max_index, keeping every engine
  under the DMA roofline.  Only the per-row argmax indices leave the
  device (128 KB/core vs 4 MB for a one-hot), so steady-state HBM traffic
  is the 33.6 MB/core input floor; measured ~92 us/pass = the pure-DMA
  floor of this platform (load-only measures the same).

Host: the inherently-sequential greedy over the first 1024 rows (tiny), the
one-hot scatter from the device argmax indices, then patch the first rows.
"""

import numpy as np

HW = 32768
J = 1024
N_CORES = 8
ROWS_PER_CORE = HW // N_CORES  # 4096
P = 128  # SBUF partitions
R = 2    # rows per partition per chunk (2 MB chunks; deeper pipeline wins
         # over larger descriptors: 91.6 us vs 93.8 us for r=4 measured)
N_CHUNKS = ROWS_PER_CORE // (P * R)  # 16
NSEG = N_CHUNKS * R  # 32 row segments per partition per pass

_NC_CACHE = {}


ADD_ENG = "g"  # "g": w = mat+noise on gpsimd/Pool, "v": on the vector engine


def _build_nc(r: int = R, bufs: int = 6, repeat: int = 1, add_eng: str = ADD_ENG):
    """Per-core Bass module: argmax(mat + noise) per row.

    Input "mn" is [2, rows_per_core, j] fp32 — mat stacked with noise (one
    tensor so each chunk loads with a single DMA instruction / single
    semaphore: TRN2 compute instructions can carry only one sync wait).
    Output "out" is [P, 8 * nseg] u32: the raw max_index results (8 slots
    per segment, slot 0 is the argmax; host takes every 8th column).
    """
    import concourse.bacc as bacc
    import concourse.mybir as mybir
    from concourse.tile import TileContext

    chunk_rows = P * r
    assert ROWS_PER_CORE % chunk_rows == 0
    n_chunks = ROWS_PER_CORE // chunk_rows
    nseg = n_chunks * r
    f32 = mybir.dt.float32
    u32 = mybir.dt.uint32

    # Bacc (not raw Bass): its finalize() runs generate_event_semaphores,
    # which splits multi-sem waits — TRN2 instructions carry at most one.
    nc = bacc.Bacc()
    mn = nc.dram_tensor("mn", [2, ROWS_PER_CORE, J], f32, kind="ExternalInput")
    # chunk c, partition p holds rows (c*128 + p)*r .. +r-1
    mnv = mn[:, :, :].rearrange("t (c p r) m -> c p t (r m)", p=P, r=r)
    out = nc.dram_tensor("out", [P, 8 * nseg], u32, kind="ExternalOutput")

    with TileContext(nc) as tc:
        with (
            tc.tile_pool(name="mnp", bufs=bufs) as mnpool,
            tc.tile_pool(name="wp", bufs=2) as wpool,
            tc.tile_pool(name="small", bufs=2) as spool,
        ):
            for _ in range(repeat):
                ix8 = spool.tile([P, 8 * nseg], u32, tag="idx")
                for c in range(n_chunks):
                    tmn = mnpool.tile([P, 2, r * J], f32, tag="mn")
                    nc.sync.dma_start(tmn[:, :, :], mnv[c])
                    w = wpool.tile([P, r * J], f32, tag="w")
                    mx = spool.tile([P, r], f32, tag="mx")
                    adder = nc.gpsimd if add_eng == "g" else nc.vector
                    adder.tensor_add(w[:], tmn[:, 0, :], tmn[:, 1, :])
                    nc.vector.tensor_reduce(
                        mx[:], w[:].rearrange("p (r m) -> p r m", r=r),
                        mybir.AxisListType.X, mybir.AluOpType.max,
                    )
                    for s in range(r):
                        g = c * r + s
                        nc.vector.max_index(
                            ix8[:, 8 * g : 8 * g + 8],
                            mx[:, s : s + 1].broadcast_to([P, 8]),
                            w[:, s * J : (s + 1) * J],
                        )
                # ACT HWDGE ring is idle (inputs go via the SP ring) and the
                # Pool/Q7s are busy with the adds — keep the out DMA off both
                nc.scalar.dma_start(out[:, :], ix8[:])
    nc.finalize()
    return nc


def _get_nc(r: int = R, bufs: int = 6, repeat: int = 1, add_eng: str = ADD_ENG):
    key = (r, bufs, repeat, add_eng)
    if key not in _NC_CACHE:
        _NC_CACHE[key] = _build_nc(*key)
    return _NC_CACHE[key]


_RUNNER_CACHE = {}


def _make_runner(r: int = R, bufs: int = 6, repeat: int = 1, add_eng: str = ADD_ENG):
    """Cached runner around run_bass_kernel_spmd.

    The first call goes through run_bass_kernel_spmd (the supported axon/PJRT
    path); during it we capture the jitted SPMD callable that
    run_bass_via_pjrt builds internally, so subsequent calls (and timing
    loops) reuse the compiled executable instead of re-tracing/re-compiling
    (run_bass_via_pjrt creates a fresh jit closure per invocation).
    """
    key = (r, bufs, repeat, add_eng)
    if key in _RUNNER_CACHE:
        return _RUNNER_CACHE[key]

    import jax
    from concourse.bass_utils import run_bass_kernel_spmd

    nc = _get_nc(r, bufs, repeat, add_eng)
    state = {"fn": None}

    def runner(mn_global: np.ndarray) -> np.ndarray:
        """mn_global: (2*N_CORES, ROWS_PER_CORE, J) per-core [mat, noise]
        pairs. Returns (N_CORES, P, 8*nseg) u32 raw index output."""
        if state["fn"] is None:
            per = mn_global.shape[0] // N_CORES
            in_maps = [{"mn": mn_global[per * k : per * (k + 1)]} for k in range(N_CORES)]
            orig_jit = jax.jit

            def capturing_jit(f, *a, **kw):
                g = orig_jit(f, *a, **kw)
                if "donate_argnums" in kw and kw.get("keep_unused"):
                    state["fn"] = g
                return g

            jax.jit = capturing_jit
            try:
                res = run_bass_kernel_spmd(nc, in_maps, core_ids=list(range(N_CORES)))
            finally:
                jax.jit = orig_jit
            out = np.stack([r_["out"] for r_ in res.results], axis=0)
            state["out_np_dtype"] = out.dtype
            state["out_shape"] = out.shape
            return out
        outs = state["fn"](mn_global, np.zeros(state["out_shape"], state["out_np_dtype"]))
        out = outs[0] if isinstance(outs, (tuple, list)) else outs
        return np.asarray(out)

    runner.state = state
    _RUNNER_CACHE[key] = runner
    return runner


def stack_inputs(mat: np.ndarray, noise: np.ndarray) -> np.ndarray:
    """Global (2*N_CORES, ROWS_PER_CORE, J): per-core [mat_shard, noise_shard]
    pairs along axis 0, so a P("core") shard is exactly the NEFF's (2, rows, J)
    "mn" tensor."""
    m3 = mat.reshape(N_CORES, ROWS_PER_CORE, J)
    n3 = noise.reshape(N_CORES, ROWS_PER_CORE, J)
    return np.stack([m3, n3], axis=1).reshape(2 * N_CORES, ROWS_PER_CORE, J)


def decode_idx(raw: np.ndarray, r: int = R) -> np.ndarray:
    """raw: (N_CORES, P, 8*nseg) u32 -> (HW,) per-row argmax.

    Segment g = c*r + s of partition p on core k holds row
    ((k*n_chunks + c)*P + p)*r + s; slot 0 of each 8-wide group is the
    argmax."""
    n_chunks = ROWS_PER_CORE // (P * r)
    a = raw.reshape(N_CORES, P, n_chunks, r, 8)[..., 0]
    return np.ascontiguousarray(a.transpose(0, 2, 1, 3)).reshape(HW)


def _greedy_select(w_first: np.ndarray) -> np.ndarray:
    """Sequential greedy: row r takes the available joint with max w[r].

    Equivalent to the reference's scan over descending top-k indices.
    """
    n = w_first.shape[0]
    avail = np.ones(n, dtype=bool)
    sel = np.empty(n, dtype=np.int64)
    neg_inf = np.float32(-np.inf)
    for r in range(n):
        row = np.where(avail, w_first[r], neg_inf)
        s = int(np.argmax(row))
        sel[r] = s
        avail[s] = False
    return sel


def run_device(mat: np.ndarray, noise: np.ndarray) -> np.ndarray:
    """Shard row-wise over 8 cores, run the Bass kernel, return (HW,) argmax."""
    runner = _make_runner()
    raw = runner(stack_inputs(mat, noise))
    return decode_idx(np.asarray(raw))


def kernel(sgt_trans_mat, gumbel_noise, use_gumbel_noise=1, is_training=1,
           temperature=30):
    mat = np.ascontiguousarray(np.asarray(sgt_trans_mat, dtype=np.float32))
    assert mat.shape == (HW, J), mat.shape
    training = bool(int(np.asarray(is_training)))
    use_g = training and bool(int(np.asarray(use_gumbel_noise)))
    if use_g:
        noise = np.ascontiguousarray(np.asarray(gumbel_noise, dtype=np.float32))
    else:
        # selection order falls back to mat itself; temperature never matters
        noise = np.zeros_like(mat)

    sel_tail = run_device(mat, noise)  # (HW,) per-row argmax of mat+noise

    # Host-side greedy over the first J rows (inherently sequential, tiny).
    w_first = mat[:J] + noise[:J]  # same IEEE fp32 add as the device
    sel = np.concatenate([_greedy_select(w_first), sel_tail[J:]])

    # out = stop_grad(one_hot(sel)) - stop_grad(mat) + mat == exact one-hot
    out = np.zeros((HW, J), dtype=np.float32)
    out[np.arange(HW), sel] = np.float32(1.0)
    return out
